# revision 56
# baseline (speedup 1.0000x reference)
"""Multi-head self-attention (B=2, T=2048, C=1024, H=16, RoPE, causal) on 8 trn2 cores.

Sharding: data-parallel over batch (2) x tensor-parallel over head groups (4).
Core c handles batch c//4, heads (c%4)*4 .. +3.  Each core computes its
4 heads' attention output and a partial out-projection (contraction over its
256 head-dims); the host sums the 4 partials per batch.

Layout/schedule per core (bf16 compute, f32 PSUM accumulate):
  - x is transposed on the HOST (x^T [C, T] staged in DRAM) - no PE
    transposes at all; x^T tiles DMA straight into SBUF.  All inputs are
    host-converted to bf16 (1 cycle/row matmuls, half the DMA/SBUF traffic,
    no <256-free-size penalty); rel err ~4e-3 vs the 2e-2 gate.
  - QKV projection emits q/k in per-head [evens(32); odds(32)] partition
    order (host-permuted w columns).  RoPE: Act copies PSUM->bf16 SBUF,
    then 6 DVE tensor_tensor ops (all-bf16 SBUF hits the DVE 2x mode).
    Rotated halves stage in one [128, 2, 1024] tile per pair; a single
    SBUF->SBUF DMA per (head, 2 qtrs) merges into per-head-contiguous
    q^T/k^T (dims interleave [e0,o0,e1,...] - q and k share the order).
  - Scores computed transposed (S^T[k, q]) so no P-transposes are needed;
    2 k-tiles x 2 heads per pipeline unit, heads on the T0/T8 PE row-tiles.
  - v slots are 128 wide: cols 0:64 = dims, 64:128 = ones.  The PV matmul
    then lands the softmax denominator on PSUM partitions 64:127 for free
    (matmul cost = moving free size, which is unchanged).
  - Normalization: numerators are evicted UNNORMALIZED to a_sb (bf16) and
    denominators staged to a [128,1024] tile per (head-pair, 2-qc block);
    1/d = exp(-ln(d)) batched on Act (ln/exp share one activation table
    set; the exact DVE reciprocal costs ~6 cycles/elem), then one in-place
    DVE multiply.  The out-projection bursts one unit later in the ps_o
    bank window, when the normalize chain has drained.
  - era-2 is software-pipelined at depth 3 with the PV pop emitted BEFORE
    the next scores: the in-order PE queue always has ready matmuls in
    hand while cross-engine semaphores (exp, PSUM-ring WARs) clear.  The
    same principle sets every pool's ring depth (ps_proj=6 of the 8 PSUM
    banks in era 1, ps_s=3 + ps_o=2 + psy window in era 2).
"""
import sys
import math

sys.path.insert(0, "/opt/trn_rl_repo")

import numpy as np
import ml_dtypes

B, T, C, H, D = 2, 2048, 1024, 16, 64
HG = H // 4            # 4 heads per core
NCORES = 8
NKC = C // 128         # 8 contraction chunks
NQTR = T // 512        # 4 t-quarters
NKT = T // 128         # 16 k-tiles
ROPE_BASE = 10000.0

_BUILT = None


# ---------------------------------------------------------------------------
# Toolchain workaround: this walrus build accepts at most ONE semaphore wait
# per instruction.  Tile's exit drain carries one wait per outstanding proc,
# and stage-1B can attach several waits to compute/DMA instructions.  We
# (a) replace the exit drain with a chain of single-wait drains, and
# (b) post-process the module, hoisting extra waits onto same-engine nops.
# ---------------------------------------------------------------------------

def _apply_tile_patch():
    import bass_rust
    import concourse.tile as tile
    from concourse.vector_clock import ScopedClock

    def _patched_drain_and_barrier(self, tick_clock, wait_clock):
        nc = self.nc
        probe = nc.sync.drain()
        wait_clock.add_sem_waits(probe.ins, ScopedClock({None: tick_clock.global_clock}))
        si = probe.ins.sync_info
        waits = list(si.on_wait) if si is not None else []
        probe.ins.sync_info = None
        name2sem = {s.name: s for s in wait_clock.sems.allocated().values()}
        for w in waits:
            d = nc.sync.drain()
            bass_rust.wait_op(d.ins, name2sem[w.ant_name], w.wait_value, "sem-ge", False)
        nc.all_engine_barrier()
        popped = nc._tile_sem_poison_stack.pop()
        assert popped is self._sem_poison
        nc.clear_and_free_semaphores(list(self.sems.allocated().values()))
        nc.all_engine_barrier()

    tile.TileContext._drain_and_barrier = _patched_drain_and_barrier


def _split_multi_waits(nc):
    import bass_rust
    import concourse.mybir as mybir

    ctr = 0
    for fn in nc.m.functions:
        for blk in fn.blocks:
            il = blk.instructions
            new = []
            changed = False
            for inst in il:
                si = inst.sync_info
                waits = list(si.on_wait) if si is not None else []
                if len(waits) > 1:
                    changed = True
                    for w in waits[:-1]:
                        nop = mybir.InstNoOp(name=f"I-waitsplit-{ctr}", ins=[], outs=[])
                        ctr += 1
                        nop.engine = inst.engine
                        nop.sync_info = bass_rust.SyncInfo(on_wait=[w], on_update=[])
                        new.append(nop)
                    inst.sync_info = bass_rust.SyncInfo(
                        on_wait=[waits[-1]], on_update=list(si.on_update)
                    )
                new.append(inst)
            if changed:
                blk.instructions = new


# ---------------------------------------------------------------------------
# Kernel builder (per-core program; identical on all 8 cores)
# ---------------------------------------------------------------------------

def build_nc(split_waits=True, loop_iters=None, phases=(1, 2), exp_cols=1024,
             dup_scores=False, x_bufs=3, rope_bufs=3, depth=3, pt_bufs=8,
             proj_bufs=6, y_bf16=False, tail_on_act=False, burst_delay=True, yt_bufs=5, y_on_act=False, split_eo=True, dn_bufs=2, v_bufs=2):
    _apply_tile_patch()
    import concourse.bass as bass
    import concourse.tile as tile
    import concourse.mybir as mybir
    from contextlib import nullcontext

    dt = mybir.dt
    f32, bf16 = dt.float32, dt.bfloat16
    Exp = mybir.ActivationFunctionType.Exp
    Ln = mybir.ActivationFunctionType.Ln
    MUL, SUB, ADD, DIV = (mybir.AluOpType.mult, mybir.AluOpType.subtract,
                          mybir.AluOpType.add, mybir.AluOpType.divide)

    nc = bass.Bass()
    xT_d = nc.dram_tensor("xT", [C, T], bf16, kind="ExternalInput")
    w_d = nc.dram_tensor("w", [C, 768], bf16, kind="ExternalInput")
    wo_d = nc.dram_tensor("wo", [256, C], bf16, kind="ExternalInput")
    cs_d = nc.dram_tensor("cs", [128, T], bf16, kind="ExternalInput")
    sn_d = nc.dram_tensor("sn", [128, T], bf16, kind="ExternalInput")
    tri_d = nc.dram_tensor("tri", [128, 128], bf16, kind="ExternalInput")
    y_dt = bf16 if y_bf16 else f32
    y_d = nc.dram_tensor("y", [T, C], y_dt, kind="ExternalOutput")

    with tile.TileContext(nc) as tc:
      loop_cm = (tc.For_i(0, loop_iters, 1,
                          hint_engines=(mybir.EngineType.PE, mybir.EngineType.Activation,
                                        mybir.EngineType.DVE, mybir.EngineType.SP,
                                        mybir.EngineType.Pool))
                 if loop_iters else nullcontext())
      with loop_cm:
        with (
            # ---- persistent pools (live across all phases)
            tc.tile_pool(name="persist", bufs=1) as persist,
            tc.tile_pool(name="qkT", bufs=1) as qkT_pool,
            tc.tile_pool(name="asb", bufs=1) as asb_pool,
        ):
            # per-head-contiguous rotated q^T/k^T: tile [128, T] = 2 heads
            qT = [qkT_pool.tile([128, T], bf16, tag=f"qT{i}", name=f"qT{i}") for i in range(2)]
            kT = [qkT_pool.tile([128, T], bf16, tag=f"kT{i}", name=f"kT{i}") for i in range(2)]
            # v in (t, d) layout; per-head slot 128 wide: [0:64)=dims, [64:128)=ones
            v_sb = persist.tile([128, NKT, 4 * 128], bf16, tag="v")
            wo_sb = persist.tile([128, 2, C], bf16, tag="wo")
            tri_sb = persist.tile([128, 128], bf16, tag="tri")
            cs_sb = persist.tile([128, T], bf16, tag="cs")
            sn_sb = persist.tile([128, T], bf16, tag="sn")
            a_sb = [asb_pool.tile([128, T], bf16, tag=f"a{i}", name=f"a{i}") for i in range(2)]

            # ones columns of v (cols 64:128 of each 128-wide head slot):
            # memset the whole tile to 1.0; projection evicts cols 0:64.
            v4 = v_sb[:].rearrange("p kt (h c) -> p kt h c", h=4)
            nc.gpsimd.memset(v_sb[:], 1.0)

            # big table loads, spread across DMA rings
            nc.scalar.dma_start(wo_sb[:], wo_d[:].rearrange("(kc p) c -> p kc c", p=128))
            nc.scalar.dma_start(tri_sb[:], tri_d[:])
            nc.sync.dma_start(cs_sb[:], cs_d[:])
            nc.sync.dma_start(sn_sb[:], sn_d[:])

            # ================= era 1: projections + RoPE =================
            with (
                tc.tile_pool(name="w", bufs=1) as w_pool,
                tc.tile_pool(name="xload", bufs=x_bufs) as x_pool,
                tc.tile_pool(name="rope", bufs=rope_bufs) as rope_pool,
                tc.tile_pool(name="ps_proj", bufs=proj_bufs, space="PSUM") as ps_proj,
                tc.tile_pool(name="ps_v", bufs=v_bufs, space="PSUM") as ps_v,
            ):
                w_sb = w_pool.tile([128, NKC, 768], bf16, tag="w")
                for half in range(2):
                    nc.scalar.dma_start(
                        w_sb[:, half * 4:(half + 1) * 4, :],
                        w_d[half * 512:(half + 1) * 512, :].rearrange("(kc p) f -> p kc f", p=128))

                stage = {}
                for qtr in range(NQTR):
                    # ---- x^T for this 512-col slab (direct DMA, no transposes)
                    xT_q = x_pool.tile([128, NKC, 512], bf16, tag="xTq")
                    for half in range(2):
                        nc.sync.dma_start(
                            xT_q[:, half * 4:(half + 1) * 4, :],
                            xT_d[half * 512:(half + 1) * 512, qtr * 512:(qtr + 1) * 512]
                            .rearrange("(kc p) t -> p kc t", p=128))

                    # ---- QK projection + RoPE (pairs: (QE,QO) then (KE,KO))
                    cs_c = cs_sb[:, qtr * 512:(qtr + 1) * 512]
                    sn_c = sn_sb[:, qtr * 512:(qtr + 1) * 512]
                    for pair in range(2):          # 0: Q, 1: K
                        m_e, m_o = 2 * pair, 2 * pair + 1
                        ps_e = ps_proj.tile([128, 512], f32, tag="proj", name="ps_e")
                        ps_o = ps_proj.tile([128, 512], f32, tag="proj", name="ps_o")
                        if split_eo:
                            # finish the evens' accumulation first so the Act
                            # rope-copy overlaps the odds' matmuls
                            for kc in range(NKC):
                                nc.tensor.matmul(ps_e[:], w_sb[:, kc, m_e * 128:(m_e + 1) * 128],
                                                 xT_q[:, kc, :], start=(kc == 0), stop=(kc == NKC - 1))
                            for kc in range(NKC):
                                nc.tensor.matmul(ps_o[:], w_sb[:, kc, m_o * 128:(m_o + 1) * 128],
                                                 xT_q[:, kc, :], start=(kc == 0), stop=(kc == NKC - 1))
                        else:
                            for kc in range(NKC):
                                nc.tensor.matmul(ps_e[:], w_sb[:, kc, m_e * 128:(m_e + 1) * 128],
                                                 xT_q[:, kc, :], start=(kc == 0), stop=(kc == NKC - 1))
                                nc.tensor.matmul(ps_o[:], w_sb[:, kc, m_o * 128:(m_o + 1) * 128],
                                                 xT_q[:, kc, :], start=(kc == 0), stop=(kc == NKC - 1))
                        # PSUM f32 -> SBUF bf16 on Act, so the DVE rope ops are
                        # all-bf16 SBUF (2x DVE mode) and Act shares era-1 load
                        eb = rope_pool.tile([128, 512], bf16, tag="eb")
                        ob = rope_pool.tile([128, 512], bf16, tag="ob")
                        nc.scalar.copy(eb[:], ps_e[:])
                        nc.scalar.copy(ob[:], ps_o[:])
                        t1 = rope_pool.tile([128, 512], bf16, tag="t1")
                        t2 = rope_pool.tile([128, 512], bf16, tag="t2")
                        # rotated output staged over 2 quarters in ONE tile
                        # ([128, 2, 1024]: dim 0 = evens/odds) so the merge
                        # into q^T/k^T is a single DMA per head
                        if qtr % 2 == 0:
                            stage[pair] = rope_pool.tile([128, 2, 1024], bf16,
                                                         tag=f"eo{pair}", name=f"eo{pair}")
                        eo = stage[pair]
                        hb = (qtr % 2) * 512
                        ev = eo[:, 0, hb:hb + 512]
                        od = eo[:, 1, hb:hb + 512]
                        nc.vector.tensor_tensor(t1[:], eb[:], cs_c, MUL)
                        nc.vector.tensor_tensor(t2[:], ob[:], sn_c, MUL)
                        nc.vector.tensor_tensor(ev, t1[:], t2[:], SUB)
                        nc.vector.tensor_tensor(t1[:], eb[:], sn_c, MUL)
                        nc.vector.tensor_tensor(t2[:], ob[:], cs_c, MUL)
                        nc.vector.tensor_tensor(od, t1[:], t2[:], ADD)

                    # ---- merge into per-head-contiguous q^T/k^T every 2 qtrs.
                    # src [32, 2, 1024] -> dst [64, 1024] interleaves each
                    # head's dims as [e0,o0,e1,o1,...]; q and k share the
                    # order, so scores are unaffected.
                    if qtr % 2 == 1:
                        sl = slice((qtr - 1) * 512, (qtr + 1) * 512)
                        for pair in range(2):
                            dstT = qT if pair == 0 else kT
                            eo = stage[pair]
                            for h in range(4):
                                h2, hh = h // 2, h % 2
                                r0 = hh * 64
                                nc.sync.dma_start(dstT[h2][r0:r0 + 64, sl],
                                                  eo[h * 32:(h + 1) * 32, :, :])

                    # ---- V projection (t-on-partition layout)
                    for tl in range(4):
                        psv = ps_v.tile([128, 256], f32, tag="v")
                        for kc in range(NKC):
                            nc.tensor.matmul(psv[:], xT_q[:, kc, tl * 128:(tl + 1) * 128],
                                             w_sb[:, kc, 512:768], start=(kc == 0), stop=(kc == NKC - 1))
                        kt = qtr * 4 + tl
                        nc.scalar.copy(v4[:, kt, :, 0:64], psv[:].rearrange("p (h d) -> p h d", h=4))


            # ============ era 2: attention + per-chunk out-projection ============
            if 2 not in phases:
                pass
            else:
              with (
                  tc.tile_pool(name="pt", bufs=pt_bufs) as pt_pool,
                  tc.tile_pool(name="yout", bufs=yt_bufs) as y_pool,
                  tc.tile_pool(name="dn", bufs=dn_bufs) as dn_pool,
                  tc.tile_pool(name="ps_s", bufs=3, space="PSUM") as ps_s_pool,
                  tc.tile_pool(name="ps_o", bufs=1, space="PSUM") as ps_o_pool,
              ):
                  # Flat software pipeline over units (qc, hp, ki) - ONE
                  # k-tile per unit, BOTH heads packed in one [128,1024]
                  # scores tile (cols hh*512..).  Same matmul/exp counts as
                  # 2-k-tile units, but the 3-deep ps_s ring then spans 3
                  # units instead of 1.5, so ring-reuse WARs never expose
                  # cross-engine semaphore latency on the in-order PE queue.
                  units = [(qc, hp, ki2)
                           for qc in range(NQTR)
                           for hp in range(2)
                           for ki2 in range((qc + 1) * 2)]

                  ps_o_cur = {}

                  def emit_scores(qc, hp, ki2):
                      ki0 = 2 * ki2
                      # S^T for BOTH heads, interleaving the T0/T8 row-tiles
                      # so both halves of the PE array overlap
                      ps_s2 = [ps_s_pool.tile([128, 1024], f32, tag="s", name=f"ps_s{hh}")
                               for hh in range(2)]
                      for half in range(2):
                          ki = ki0 + half
                          soff = max(0, ki * 128 - qc * 512) if ki // 4 == qc else 0
                          for hh in range(2):
                              r0 = hh * 64
                              for _rep in range(2 if dup_scores else 1):
                                  nc.tensor.matmul(
                                      ps_s2[hh][:, half * 512 + soff:(half + 1) * 512],
                                      kT[hp][r0:r0 + 64, ki * 128:(ki + 1) * 128],
                                      qT[hp][r0:r0 + 64, qc * 512 + soff:(qc + 1) * 512],
                                      start=True, stop=True, skip_group_check=True)
                      return ps_s2

                  def emit_exp(qc, hp, ki2, ps_s2):
                      ki0 = 2 * ki2
                      pts = []
                      for hh in range(2):
                          ps_s = ps_s2[hh]
                          pt = pt_pool.tile([128, 1024], bf16, tag="pt", name=f"pt{hh}")
                          pts.append(pt)
                          # one full-width exp; causal masking of diagonal
                          # tiles happens AFTER on Pool (memset) + DVE (tri)
                          nc.scalar.activation(pt[:, 0:exp_cols], ps_s[:, 0:exp_cols],
                                               Exp, scale=0.125)
                          for half in range(2):
                              ki = ki0 + half
                              if ki // 4 != qc:
                                  continue        # fully below the diagonal
                              base = half * 512
                              off = ki * 128 - qc * 512
                              if off > 0:
                                  nc.gpsimd.memset(pt[:, base:base + off], 0.0)
                              nc.vector.tensor_tensor(
                                  pt[:, base + off:base + off + 128],
                                  pt[:, base + off:base + off + 128],
                                  tri_sb[:], MUL)
                      return pts

                  def emit_pv(qc, hp, ki2, pts):
                      ki0 = 2 * ki2
                      nkt_q = (qc + 1) * 4
                      ps_o = ps_o_cur[(qc, hp)]
                      for hh in range(2):
                          h = hp * 2 + hh
                          for half in range(2):
                              ki = ki0 + half
                              soff = (max(0, ki * 128 - qc * 512)
                                      if (ki // 4 == qc and ki != 0) else 0)
                              nc.tensor.matmul(
                                  ps_o[hh][:, soff:512],
                                  v_sb[:, ki, h * 128:(h + 1) * 128],
                                  pts[hh][:, half * 512 + soff:(half + 1) * 512],
                                  start=(ki == 0), stop=(ki == nkt_q - 1),
                                  skip_group_check=True)

                  dn_cur = {}

                  def emit_tail(qc, hp):
                      # evict PSUM: UNNORMALIZED numerators -> a_sb (bf16),
                      # denominator rows -> the block's dn staging tile.  The
                      # exact DVE reciprocal costs ~6 cycles/elem, so instead
                      # compute 1/d = exp(-ln(d)) on Act, batched [128,1024]
                      # per (hp, 2-qc block): ln and exp share an activation
                      # table set, and Act has slack that DVE doesn't.
                      ps_o = ps_o_cur.pop((qc, hp))
                      if qc % 2 == 0:
                          dn_cur[hp] = dn_pool.tile([128, 1024], f32,
                                                    tag=f"dn{hp}", name=f"dn{hp}")
                      dn = dn_cur[hp]
                      cp_eng = nc.scalar.copy if tail_on_act else nc.vector.tensor_copy
                      for hh in range(2):
                          cp_eng(
                              a_sb[hp][hh * 64:(hh + 1) * 64, qc * 512:(qc + 1) * 512],
                              ps_o[hh][0:64, :])
                          cp_eng(
                              dn[hh * 64:(hh + 1) * 64, (qc % 2) * 512:(qc % 2 + 1) * 512],
                              ps_o[hh][64:128, :])
                      if qc % 2 == 1:
                          # normalize this hp's finished 2-qc block in place
                          cols = slice((qc - 1) * 512, (qc + 1) * 512)
                          dn = dn_cur.pop(hp)
                          lnd = dn_pool.tile([128, 1024], f32, tag="lnd")
                          rcp = dn_pool.tile([128, 1024], f32, tag="rcp")
                          nc.scalar.activation(lnd[:], dn[:], Ln)
                          nc.scalar.activation(rcp[:], lnd[:], Exp, scale=-1.0)
                          nc.vector.tensor_tensor(
                              a_sb[hp][:, cols], a_sb[hp][:, cols], rcp[:], MUL)
                          if hp == 1:
                              if burst_delay:
                                  burst_q.append(qc)
                              else:
                                  emit_block_outproj(qc)

                  def emit_block_outproj(qc):
                      # burst the block's out-projection while ps_o banks are
                      # free (between the last PV of this block's groups and
                      # the first popped PV of the next)
                      for ti in range((qc - 1) * 4, (qc + 1) * 4):
                          yt = y_pool.tile([128, 1024], y_dt, tag="yt")
                          for ncol in range(2):
                              psy = ps_o_pool.tile([128, 512], f32,
                                                   tag=f"o{ncol}", name="psy")
                              for kc2 in range(2):
                                  nc.tensor.matmul(psy[:], a_sb[kc2][:, ti * 128:(ti + 1) * 128],
                                                   wo_sb[:, kc2, ncol * 512:(ncol + 1) * 512],
                                                   start=(kc2 == 0), stop=(kc2 == 1))
                              nc.vector.tensor_copy(yt[:, ncol * 512:(ncol + 1) * 512], psy[:])
                          y_eng = nc.scalar if y_on_act else nc.sync
                          y_eng.dma_start(y_d[ti * 128:(ti + 1) * 128, :], yt[:])

                  # depth-2 software pipeline: PV runs two units behind its
                  # scores/exp, so the PV->exp semaphore is long-propagated
                  # by the time the in-order PE queue reaches the PV matmuls.
                  DEPTH = depth
                  pending = []        # [(qc, hp, ki2, pts), ...]

                  def pop_pending():
                      pqc, php, pki2, ppts = pending.pop(0)
                      emit_pv(pqc, php, pki2, ppts)
                      if pki2 == (pqc + 1) * 2 - 1:
                          emit_tail(pqc, php)

                  burst_q = []
                  for (qc, hp, ki2) in units:
                      if ki2 == 0:
                          ps_o_cur[(qc, hp)] = [
                              ps_o_pool.tile([128, 512], f32, tag=f"o{hh}", name=f"pso{hh}")
                              for hh in range(2)]
                      # delayed out-projection burst: one unit after its tail,
                      # so the ln/exp/mul normalize chain is already done
                      if burst_q:
                          emit_block_outproj(burst_q.pop(0))
                      # pop BEFORE scores: the PE then executes PV(w-2) while
                      # the scores' ps_s ring-reuse semaphore clears
                      if len(pending) >= DEPTH:
                          pop_pending()
                      ps_s2 = emit_scores(qc, hp, ki2)
                      pts = emit_exp(qc, hp, ki2, ps_s2)
                      pending.append((qc, hp, ki2, pts))
                  while pending:
                      pop_pending()
                  while burst_q:
                      emit_block_outproj(burst_q.pop(0))

    if split_waits:
        _split_multi_waits(nc)
    return nc


# ---------------------------------------------------------------------------
# Host-side sharding / gather
# ---------------------------------------------------------------------------

def _rope_tables():
    inv_freq = (1.0 / (ROPE_BASE ** (np.arange(0, D, 2, dtype=np.float32) / D))).astype(np.float32)
    ang = (np.arange(T, dtype=np.float32)[:, None] * inv_freq[None, :]).astype(np.float32)  # (T, 32)
    cos, sin = np.cos(ang), np.sin(ang)
    idx = np.arange(128) % 32
    return np.ascontiguousarray(cos[:, idx].T), np.ascontiguousarray(sin[:, idx].T)  # (128, T)


def _perm_cols(g):
    """w_qkv column order for core group g: [QE|QO|KE|KO|V]."""
    cols = []
    for base, par in ((0, 0), (0, 1), (C, 0), (C, 1)):      # QE, QO, KE, KO
        for hl in range(4):
            hg = g * 4 + hl
            for i in range(32):
                cols.append(base + hg * 64 + 2 * i + par)
    for hl in range(4):
        hg = g * 4 + hl
        for d_ in range(64):
            cols.append(2 * C + hg * 64 + d_)
    return np.asarray(cols)


def make_in_maps(x, w_qkv, w_out):
    bf = ml_dtypes.bfloat16
    x = np.asarray(x, np.float32)
    w_qkv = np.asarray(w_qkv, np.float32)
    w_out = np.asarray(w_out, np.float32)
    cs, sn = _rope_tables()
    cs, sn = cs.astype(bf), sn.astype(bf)
    tri = np.tril(np.ones((128, 128), np.float32)).T.astype(bf)  # tri[k, q] = 1 iff q >= k
    xT = [np.ascontiguousarray(x[b].T).astype(bf) for b in range(B)]
    in_maps = []
    for c in range(NCORES):
        b, g = c // 4, c % 4
        in_maps.append({
            "xT": xT[b],
            "w": np.ascontiguousarray(w_qkv[:, _perm_cols(g)]).astype(bf),
            "wo": np.ascontiguousarray(w_out[g * 256:(g + 1) * 256, :]).astype(bf),
            "cs": cs, "sn": sn, "tri": tri,
        })
    return in_maps


def kernel(x, w_qkv, w_out):
    global _BUILT
    from concourse.bass_utils import run_bass_kernel_spmd

    if _BUILT is None:
        _BUILT = build_nc()
    in_maps = make_in_maps(x, w_qkv, w_out)
    res = run_bass_kernel_spmd(_BUILT, in_maps, core_ids=list(range(NCORES)))
    out = np.zeros((B, T, C), np.float32)
    for c in range(NCORES):
        out[c // 4] += res.results[c]["y"]
    return out
